# revision 6
# baseline (speedup 1.0000x reference)
"""Trainium2 kernel for nn_K_graph (gnn_message_passing).

Strategy: per sharding_hint, the C=32 per-column subgraphs are distributed
across 8 NeuronCores (4 per core). Each graph c only contains the rows whose
top-K includes column c (counts range ~16..680, mean 256), so the device
kernel works on COMPACTED per-graph node lists instead of the full B=1024:
graphs are sorted by node count into four size slots [768, 384, 384, 256]
(one graph per slot per core; identical instruction stream on all cores).

Per graph (compact size P, nb = P/128 row blocks):
  S = pmc pmc^T            K=32 matmuls; diagonal suppressed by accumulating
                           a (-BIG*I) @ I matmul into the same PSUM bank
  f = exp(S)               scalar engine, PSUM->SBUF
  E = (f > 1) * f          vector stt (exactly reproduces the S>0 mask since
                           structural zeros give f == 1.0), rowsums via accum
  deg/dinv chain           tiny per-partition ops
  2 GCN layers, transposed layout: u^T[h,i] = sum_j E[j,i] ydn[j,h] computed
  with the small ydn blocks as the stationary matmul operand (E symmetric);
  the self-loop term enters the same PSUM accumulation as ydn^T Z*eye; the
  masked layernorm runs on [128, P] pair-stacked tiles (two graphs share the
  128 partitions, 64 rows of h each).

Host does the tiny front (feature embed, importance MLP, top-K) and tail
(gather + prediction MLP) plus the compaction bookkeeping.
"""
import sys, os
sys.path.insert(0, "/opt/trn_rl_repo")
import numpy as np

B, NN, NC, H, V, K = 1024, 16, 16, 64, 100, 8
C = NN + NC
NEG = -1e9
NCORE = 8
GPC = C // NCORE  # graphs per core = 4

F32 = np.float32

# compact slot layout (identical on every core)
SLOTS = [768, 384, 384, 256]
NBS = [6, 3, 3, 2]
BLKOFF = [0, 6, 9, 12]
NB_TOT = 14
COLOFF = [0, 768, 1152, 1536]
P_TOT = 1792
PAIRS = [(0, 1), (2, 3)]     # (bigger slot -> partitions 0:64, smaller -> 64:128)
PPOFF = [0, 768]             # column offset of each pair in mct / xo
PO_TOT = 1152
BIG = 50.0


# ---------------- host front (numpy mirror of reference front) -------------
def _ln_all(x, eps=1e-5):
    mu = x.mean()
    var = ((x - mu) ** 2).mean()
    return (x - mu) / np.sqrt(var + eps)


def _ln_last(x, g, b, eps=1e-5):
    mu = x.mean(-1, keepdims=True)
    var = ((x - mu) ** 2).mean(-1, keepdims=True)
    return (x - mu) / np.sqrt(var + eps) * g + b


def _front(num_data, cat_data, num_w, num_b, cat_emb, fi_w1, fi_b1, fi_g,
           fi_be, fi_w2, fi_b2, gcn1_w):
    fe_num = num_data[..., None] * num_w[None] + num_b[None]
    fe_num = _ln_all(np.maximum(fe_num.reshape(B, NN * H), 0.0))
    fe_cat = cat_emb[np.arange(NC)[None, :], cat_data]
    fe_cat = _ln_all(fe_cat.reshape(B, NC * H))
    feat = np.concatenate([fe_num, fe_cat], axis=1).astype(F32)
    fe3 = feat.reshape(B, C, H)
    h = np.maximum(fe3 @ fi_w1 + fi_b1, 0.0)
    h = _ln_last(h, fi_g, fi_be)
    imp = _ln_all((h @ fi_w2 + fi_b2)[..., 0]).astype(F32)   # [B,C]
    fe3 = (fe3 * imp[..., None]).astype(F32)
    feat = fe3.reshape(B, C * H)
    # top-K per row
    idx = np.argsort(-imp, axis=1, kind="stable")[:, :K]      # [B,K]
    mask = np.zeros((B, C), F32)
    np.put_along_axis(mask, idx, 1.0, axis=1)
    z = np.where(mask > 0, imp, NEG)
    z = z - z.max(1, keepdims=True)
    e = np.exp(z)
    p = (e / e.sum(1, keepdims=True)) * mask                  # [B,C]
    mT = mask.T.copy()                                        # [C,B]
    pm = p[None, :, :] * mT[:, :, None] * (1.0 - np.eye(C, dtype=F32))[:, None, :]
    Y1 = (feat @ gcn1_w).astype(F32)                          # [B,H]
    return fe3, idx, mT, pm.astype(F32), Y1


# ---------------- numpy middle (validation / fallback) ---------------------
def _middle_np(pm, mT, Y1, gcn1_b, gcn2_w, gcn2_b):
    xs = np.zeros((C, B, H), F32)
    for c in range(C):
        M = pm[c]                               # [B,C]
        S = (M @ M.T) * (1.0 - np.eye(B, dtype=F32))
        Ffull = np.exp(S)
        E = (S > 0).astype(F32) * Ffull
        rs = E.sum(1)
        Z = rs.sum()
        Zg = Z + (1.0 if Z <= 0 else 0.0)
        invZ = 1.0 / Zg
        m = mT[c]
        deg = rs * invZ + m
        dinv = 1.0 / np.sqrt(deg + 1.0 - m) * m
        x = Y1
        for (W, bvec) in ((None, gcn1_b), (gcn2_w, gcn2_b)):
            Yin = x if W is None else x @ W
            Ydn = dinv[:, None] * Yin
            u = E @ Ydn
            xl = dinv[:, None] * (u * invZ + m[:, None] * Ydn) + bvec
            r = np.maximum(xl, 0.0)
            rm = r * m[:, None]
            cnt = max(m.sum() * H, 1.0)
            mu = rm.sum() / cnt
            var = (rm * rm).sum() / cnt - mu * mu
            x = (r - mu) / np.sqrt(var + 1e-5)
        xs[c] = x
    return xs


# ---------------- device kernel -------------------------------------------
def _build_device():
    from concourse import bacc, tile
    import concourse.bass as bass
    import concourse.mybir as mybir
    dt = mybir.dt.float32
    ALU = mybir.AluOpType
    ACT = mybir.ActivationFunctionType
    AX = mybir.AxisListType

    nc = bacc.Bacc(None, target_bir_lowering=False, debug=False)
    pmT_d = nc.declare_dram_parameter("pmT", [32, P_TOT], dt, isOutput=False)
    y1r_d = nc.declare_dram_parameter("y1r", [128, NB_TOT, H], dt, isOutput=False)
    mrow_d = nc.declare_dram_parameter("mrow", [128, NB_TOT], dt, isOutput=False)
    mct_d = nc.declare_dram_parameter("mct", [2, PO_TOT], dt, isOutput=False)
    icnt_d = nc.declare_dram_parameter("icnt", [1, 4], dt, isOutput=False)
    w2_d = nc.declare_dram_parameter("w2rep", [128, H], dt, isOutput=False)
    b1_d = nc.declare_dram_parameter("b1bc", [128, 1], dt, isOutput=False)
    b2_d = nc.declare_dram_parameter("b2bc", [128, 1], dt, isOutput=False)
    eye_d = nc.declare_dram_parameter("eye", [128, 128], dt, isOutput=False)
    e2_d = nc.declare_dram_parameter("e2", [2, 128], dt, isOutput=False)
    e2t_d = nc.declare_dram_parameter("e2t", [128, 2], dt, isOutput=False)
    sel_d = nc.declare_dram_parameter("sel", [6, 6 * H], dt, isOutput=False)
    xo_d = nc.declare_dram_parameter("xo", [128, PO_TOT], dt, isOutput=True)

    def chunks_of(P):
        return [(c, min(c + 512, P)) for c in range(0, P, 512)]

    with tile.TileContext(nc) as tc:
        with (
            tc.tile_pool(name="const", bufs=1) as cpool,
            tc.tile_pool(name="estore", bufs=1) as epool,
            tc.tile_pool(name="work", bufs=2) as wp,
            tc.tile_pool(name="scal", bufs=3) as sp,
            tc.tile_pool(name="psS", bufs=2, space=bass.MemorySpace.PSUM) as psS,
            tc.tile_pool(name="psU", bufs=1, space=bass.MemorySpace.PSUM) as psU,
            tc.tile_pool(name="psT", bufs=2, space=bass.MemorySpace.PSUM) as psT,
        ):
            pmT_sb = cpool.tile([32, P_TOT], dt)
            y1r_sb = cpool.tile([128, NB_TOT, H], dt)
            mrow_sb = cpool.tile([128, NB_TOT], dt)
            mct_sb = cpool.tile([2, PO_TOT], dt)
            icnt_sb = cpool.tile([1, 4], dt)
            w2_sb = cpool.tile([128, H], dt)
            b1_sb = cpool.tile([128, 1], dt)
            b2_sb = cpool.tile([128, 1], dt)
            eye_sb = cpool.tile([128, 128], dt)
            e2_sb = cpool.tile([2, 128], dt)
            e2t_sb = cpool.tile([128, 2], dt)
            sel_sb = cpool.tile([6, 6 * H], dt)
            negeye = cpool.tile([128, 128], dt)
            ones_r = cpool.tile([1, 128], dt)
            ones_c = cpool.tile([128, 1], dt)
            nc.sync.dma_start(pmT_sb[:], pmT_d[:])
            nc.sync.dma_start(y1r_sb[:], y1r_d[:])
            nc.sync.dma_start(mrow_sb[:], mrow_d[:])
            nc.sync.dma_start(mct_sb[:], mct_d[:])
            nc.sync.dma_start(icnt_sb[:], icnt_d[:])
            nc.sync.dma_start(w2_sb[:], w2_d[:])
            nc.sync.dma_start(b1_sb[:], b1_d[:])
            nc.sync.dma_start(b2_sb[:], b2_d[:])
            nc.sync.dma_start(eye_sb[:], eye_d[:])
            nc.sync.dma_start(e2_sb[:], e2_d[:])
            nc.sync.dma_start(e2t_sb[:], e2t_d[:])
            nc.sync.dma_start(sel_sb[:], sel_d[:])
            nc.vector.memset(ones_r[:], 1.0)
            nc.vector.memset(ones_c[:], 1.0)
            nc.vector.tensor_scalar_mul(negeye[:], eye_sb[:], -BIG)

            def bscalar(src_11, tag):
                """broadcast [1,1] sbuf scalar -> [128,1] sbuf"""
                ps = psT.tile([128, 1], dt, tag="sm")
                nc.tensor.matmul(ps[:], ones_r[:], src_11, start=True, stop=True)
                sb = sp.tile([128, 1], dt, tag=tag)
                nc.vector.tensor_copy(sb[:], ps[:])
                return sb

            def bpair(src_12, tag):
                """[1,2] sbuf -> [128,1] sbuf with halves from cols 0/1"""
                ps1 = psT.tile([2, 128], dt, tag="sm")
                nc.tensor.matmul(ps1[:], src_12, ones_r[:], start=True, stop=True)
                v2r = sp.tile([2, 128], dt, tag="v2r")
                nc.vector.tensor_copy(v2r[:], ps1[:])
                ps2 = psT.tile([128, 1], dt, tag="sm")
                nc.tensor.matmul(ps2[:], e2_sb[:], v2r[:, 0:1], start=True,
                                 stop=True)
                sb = sp.tile([128, 1], dt, tag=tag)
                nc.vector.tensor_copy(sb[:], ps2[:])
                return sb

            # ---------------- per-slot: S, E, rowsums ----------------
            E_sb = []
            rs_sb = []
            for s in range(4):
                P, nb, off = SLOTS[s], NBS[s], COLOFF[s]
                E_s = epool.tile([128, nb, P], dt, tag=f"E{s}")
                rs_s = wp.tile([128, nb], dt, tag=f"rs{s}")
                for ib in range(nb):
                    s_ps = psS.tile([128, 768], dt, tag="sps")
                    d0 = ib * 128
                    for (c0, c1) in chunks_of(P):
                        has_diag = c0 <= d0 < c1
                        nc.tensor.matmul(
                            s_ps[:, c0:c1],
                            pmT_sb[:, off + d0:off + d0 + 128],
                            pmT_sb[:, off + c0:off + c1],
                            start=True, stop=not has_diag)
                        if has_diag:
                            nc.tensor.matmul(
                                s_ps[:, d0:d0 + 128], negeye[:], eye_sb[:],
                                start=False, stop=True, skip_group_check=True)
                    f_sb = wp.tile([128, 768], dt, tag="f")
                    nc.scalar.activation(f_sb[:, 0:P], s_ps[:, 0:P], ACT.Exp)
                    nc.vector.scalar_tensor_tensor(
                        E_s[:, ib, :], f_sb[:, 0:P], 1.0, f_sb[:, 0:P],
                        ALU.is_gt, ALU.mult,
                        accum_out=rs_s[:, ib:ib + 1])
                E_sb.append(E_s)
                rs_sb.append(rs_s)

            # ---------------- per-pair processing ----------------
            for pp, (sa, sb_) in enumerate(PAIRS):
                PA, PB = SLOTS[sa], SLOTS[sb_]
                nbA, nbB = NBS[sa], NBS[sb_]
                poff = PPOFF[pp]

                # Z totals -> [1,2]
                z_ps = psT.tile([1, 2], dt, tag="sm")
                for h, s in enumerate((sa, sb_)):
                    rsr = sp.tile([128, 1], dt, tag="rsr")
                    nc.vector.tensor_reduce(rsr[:], rs_sb[s][:], AX.X, ALU.add)
                    nc.tensor.matmul(z_ps[:, h:h + 1], rsr[:], ones_c[:],
                                     start=True, stop=True,
                                     skip_group_check=True)
                z2 = sp.tile([1, 2], dt, tag="z2")
                nc.vector.tensor_copy(z2[:], z_ps[:])
                zi = sp.tile([1, 2], dt, tag="zi")
                nc.vector.tensor_scalar(zi[:], z2[:], 0.0, None, ALU.is_le)
                zg = sp.tile([1, 2], dt, tag="zg")
                nc.vector.tensor_add(zg[:], z2[:], zi[:])
                invz2 = sp.tile([1, 2], dt, tag="invz2")
                nc.vector.reciprocal(invz2[:], zg[:])

                invzP = [bscalar(invz2[:, h:h + 1], f"invzP{pp}_{h}")
                         for h in range(2)]
                zgP = [bscalar(zg[:, h:h + 1], f"zgP{pp}_{h}")
                       for h in range(2)]

                # deg chain per graph (row layout) + eyeZ + dinvT broadcast
                dinv_g = []
                eyeZ_g = []
                dbc = wp.tile([128, 768 if pp == 0 else 384], dt,
                              tag=f"dbc{pp}")
                dbc_ps = psU.tile([128, 768], dt, tag="ubig")
                for h, s in enumerate((sa, sb_)):
                    P, nb = SLOTS[s], NBS[s]
                    degg = wp.tile([128, NB_TOT], dt, tag="degg")
                    nc.scalar.activation(degg[:, 0:nb], rs_sb[s][:],
                                         ACT.Identity, bias=1.0,
                                         scale=invzP[h][:, 0:1])
                    dsq = wp.tile([128, NB_TOT], dt, tag="dsq")
                    nc.scalar.activation(dsq[:, 0:nb], degg[:, 0:nb], ACT.Sqrt)
                    draw = wp.tile([128, NB_TOT], dt, tag="draw")
                    nc.vector.reciprocal(draw[:, 0:nb], dsq[:, 0:nb])
                    dinv = wp.tile([128, NB_TOT], dt, tag=f"dinv{pp}_{h}")
                    nc.vector.tensor_mul(
                        dinv[:, 0:nb], draw[:, 0:nb],
                        mrow_sb[:, BLKOFF[s]:BLKOFF[s] + nb])
                    dinv_g.append(dinv)
                    eyeZ = wp.tile([128, 128], dt, tag=f"eyeZ{pp}_{h}")
                    nc.vector.tensor_scalar_mul(eyeZ[:], eye_sb[:],
                                                zgP[h][:, 0:1])
                    eyeZ_g.append(eyeZ)
                    # dinv [128,nb] -> transposed broadcast rows in dbc half
                    t_ps = psT.tile([6, 128], dt, tag="sm")
                    nc.tensor.transpose(t_ps[0:nb, :], dinv[:, 0:nb], eye_sb[:])
                    dT = sp.tile([6, 128], dt, tag="dT")
                    nc.vector.tensor_copy(dT[0:nb, :], t_ps[0:nb, :])
                    for jb in range(nb):
                        nc.tensor.matmul(
                            dbc_ps[h * 64:(h + 1) * 64, jb * 128:(jb + 1) * 128],
                            sel_sb[0:nb, jb * H:(jb + 1) * H],
                            dT[0:nb, :], start=True, stop=True,
                            skip_group_check=True)
                nc.scalar.activation(dbc[0:64, 0:PA], dbc_ps[0:64, 0:PA],
                                     ACT.Copy)
                nc.scalar.activation(dbc[64:128, 0:PB], dbc_ps[64:128, 0:PB],
                                     ACT.Copy)
                if PB < PA:
                    nc.vector.memset(dbc[64:128, PB:PA], 0.0)

                # mask broadcast tile for the pair
                mbc = wp.tile([128, 768 if pp == 0 else 384], dt,
                              tag=f"mbc{pp}")
                mbc_ps = psU.tile([128, 768], dt, tag="ubig")
                for (c0, c1) in chunks_of(PA):
                    nc.tensor.matmul(mbc_ps[:, c0:c1], e2_sb[:],
                                     mct_sb[:, poff + c0:poff + c1],
                                     start=True, stop=True,
                                     skip_group_check=True)
                nc.scalar.activation(mbc[:, 0:PA], mbc_ps[:, 0:PA], ACT.Copy)

                # ---------------- two GCN layers ----------------
                x_prev = None
                for layer in range(2):
                    b_sb = b1_sb if layer == 0 else b2_sb
                    # ydn row blocks [128, nb, H] per graph
                    ydn = wp.tile([128, NB_TOT, H], dt, tag=f"ydn{pp}")
                    for h, s in enumerate((sa, sb_)):
                        nb, bo = NBS[s], BLKOFF[s]
                        for jb in range(nb):
                            if layer == 0:
                                nc.vector.tensor_scalar_mul(
                                    ydn[:, bo + jb, :], y1r_sb[:, bo + jb, :],
                                    dinv_g[h][:, jb:jb + 1])
                            else:
                                y2_ps = psT.tile([128, H], dt, tag="sm")
                                nc.tensor.matmul(
                                    y2_ps[:],
                                    x_prev[h * 64:(h + 1) * 64,
                                           jb * 128:(jb + 1) * 128],
                                    w2_sb[h * 64:(h + 1) * 64, :],
                                    start=True, stop=True)
                                nc.scalar.activation(
                                    ydn[:, bo + jb, :], y2_ps[:], ACT.Copy,
                                    scale=dinv_g[h][:, jb:jb + 1])
                    # propagation matmuls: u^T += E^T ydn + Z ydn^T
                    u_ps = psU.tile([128, 768], dt, tag="ubig")
                    for h, s in enumerate((sa, sb_)):
                        P, nb, bo = SLOTS[s], NBS[s], BLKOFF[s]
                        h0 = h * 64
                        for (c0, c1) in chunks_of(P):
                            for jb in range(nb):
                                nc.tensor.matmul(
                                    u_ps[h0:h0 + 64, c0:c1],
                                    ydn[:, bo + jb, :],
                                    E_sb[s][:, jb, c0:c1],
                                    start=(jb == 0), stop=False,
                                    skip_group_check=True)
                            n_ib = [ib for ib in range(nb)
                                    if c0 <= ib * 128 < c1]
                            for k, ib in enumerate(n_ib):
                                nc.tensor.matmul(
                                    u_ps[h0:h0 + 64, ib * 128:(ib + 1) * 128],
                                    ydn[:, bo + ib, :], eyeZ_g[h][:],
                                    start=False, stop=(k == len(n_ib) - 1),
                                    skip_group_check=True)
                    # t = invz * u   (junk tail of smaller graph zeroed)
                    t_sb = wp.tile([128, 768 if pp == 0 else 384], dt,
                                   tag=f"t{pp}")
                    nc.scalar.activation(t_sb[0:64, 0:PA], u_ps[0:64, 0:PA],
                                         ACT.Copy, scale=invzP[0][0:64, 0:1])
                    nc.scalar.activation(t_sb[64:128, 0:PB],
                                         u_ps[64:128, 0:PB],
                                         ACT.Copy, scale=invzP[1][64:128, 0:1])
                    if PB < PA:
                        nc.vector.memset(t_sb[64:128, PB:PA], 0.0)
                    w_sb = wp.tile([128, 768 if pp == 0 else 384], dt,
                                   tag=f"w{pp}")
                    nc.vector.tensor_mul(w_sb[:, 0:PA], t_sb[:, 0:PA],
                                         dbc[:, 0:PA])
                    r_sb = wp.tile([128, 768 if pp == 0 else 384], dt,
                                   tag=f"r{pp}")
                    nc.vector.tensor_scalar(r_sb[:, 0:PA], w_sb[:, 0:PA],
                                            b_sb[:, 0:1], 0.0, ALU.add,
                                            ALU.max)
                    s1 = sp.tile([128, 1], dt, tag="s1")
                    rm_sb = wp.tile([128, 768 if pp == 0 else 384], dt,
                                    tag=f"rm{pp}")
                    nc.vector.scalar_tensor_tensor(
                        rm_sb[:, 0:PA], r_sb[:, 0:PA], 1.0, mbc[:, 0:PA],
                        ALU.mult, ALU.mult, accum_out=s1[:])
                    s2 = sp.tile([128, 1], dt, tag="s2")
                    sq_sb = wp.tile([128, 768 if pp == 0 else 384], dt,
                                    tag=f"sq{pp}")
                    nc.scalar.activation(sq_sb[:, 0:PA], rm_sb[:, 0:PA],
                                         ACT.Square, accum_out=s2[:])
                    # per-graph stats: [1,2] = per-half partition sums
                    st_ps = psT.tile([1, 4], dt, tag="sm")
                    nc.tensor.matmul(st_ps[:, 0:2], s1[:], e2t_sb[:],
                                     start=True, stop=True,
                                     skip_group_check=True)
                    nc.tensor.matmul(st_ps[:, 2:4], s2[:], e2t_sb[:],
                                     start=True, stop=True,
                                     skip_group_check=True)
                    st = sp.tile([1, 4], dt, tag="st")
                    nc.vector.tensor_copy(st[:], st_ps[:])
                    mu = sp.tile([1, 2], dt, tag="mu")
                    nc.vector.tensor_mul(mu[:], st[:, 0:2],
                                         icnt_sb[:, 2 * pp:2 * pp + 2])
                    e2m = sp.tile([1, 2], dt, tag="e2m")
                    nc.vector.tensor_mul(e2m[:], st[:, 2:4],
                                         icnt_sb[:, 2 * pp:2 * pp + 2])
                    musq = sp.tile([1, 2], dt, tag="musq")
                    nc.vector.tensor_mul(musq[:], mu[:], mu[:])
                    var = sp.tile([1, 2], dt, tag="var")
                    nc.vector.tensor_sub(var[:], e2m[:], musq[:])
                    vare = sp.tile([1, 2], dt, tag="vare")
                    nc.vector.tensor_scalar_add(vare[:], var[:], 1e-5)
                    sig = sp.tile([1, 2], dt, tag="sig")
                    nc.scalar.activation(sig[:], vare[:], ACT.Sqrt)
                    rsig = sp.tile([1, 2], dt, tag="rsig")
                    nc.vector.reciprocal(rsig[:], sig[:])
                    nmr = sp.tile([1, 2], dt, tag="nmr")
                    nc.vector.scalar_tensor_tensor(nmr[:], mu[:], -1.0,
                                                   rsig[:], ALU.mult, ALU.mult)
                    rsigP = bpair(rsig[:], f"rsigP{pp}")
                    nmrP = bpair(nmr[:], f"nmrP{pp}")
                    x_sb = wp.tile([128, 768 if pp == 0 else 384], dt,
                                   tag=f"x{layer}_{pp}")
                    nc.vector.tensor_scalar(x_sb[:, 0:PA], r_sb[:, 0:PA],
                                            rsigP[:, 0:1], nmrP[:, 0:1],
                                            ALU.mult, ALU.add)
                    x_prev = x_sb
                nc.sync.dma_start(xo_d[:, poff:poff + PA], x_prev[:, 0:PA])
    nc.finalize()
    return nc


_NC_CACHE = None
_LAST_EXEC_NS = None
_LAST_TRACE_PATH = None


def _assign(counts):
    """sort graphs into (core, slot); returns list of (graph, core, slot)."""
    order = np.argsort(-counts, kind="stable")
    for s in range(4):
        if counts[order[8 * s]] > SLOTS[s]:
            return None
    asg = []
    for s in range(4):
        for r in range(NCORE):
            asg.append((int(order[8 * s + r]), r, s))
    return asg


def _middle_device(pm, mT, Y1, gcn1_b, gcn2_w, gcn2_b, counts, asg):
    global _NC_CACHE, _LAST_EXEC_NS, _LAST_TRACE_PATH
    from concourse.bass_utils import run_bass_kernel_spmd
    if _NC_CACHE is None:
        _NC_CACHE = _build_device()
    nc = _NC_CACHE

    eye = np.eye(128, dtype=F32)
    e2 = np.zeros((2, 128), F32)
    e2[0, 0:64] = 1.0
    e2[1, 64:128] = 1.0
    e2t = e2.T.copy()
    sel = np.zeros((6, 6 * H), F32)
    for jb in range(6):
        sel[jb, jb * H:(jb + 1) * H] = 1.0
    w2rep = np.concatenate([gcn2_w, gcn2_w], axis=0).astype(F32)  # [128,64]
    b1bc = np.concatenate([gcn1_b, gcn1_b])[:, None].astype(F32)  # [128,1]
    b2bc = np.concatenate([gcn2_b, gcn2_b])[:, None].astype(F32)

    idx_of = {}
    in_maps = []
    for r in range(NCORE):
        in_maps.append(dict(
            pmT=np.zeros((32, P_TOT), F32),
            y1r=np.zeros((128, NB_TOT, H), F32),
            mrow=np.zeros((128, NB_TOT), F32),
            mct=np.zeros((2, PO_TOT), F32),
            icnt=np.ones((1, 4), F32),
            w2rep=w2rep, b1bc=b1bc, b2bc=b2bc, eye=eye, e2=e2, e2t=e2t,
            sel=sel))
    for (g, r, s) in asg:
        P, nb, bo, co = SLOTS[s], NBS[s], BLKOFF[s], COLOFF[s]
        n = int(counts[g])
        idx_c = np.nonzero(mT[g] > 0)[0]
        idx_of[g] = idx_c
        im = in_maps[r]
        im["pmT"][:, co:co + n] = pm[g][idx_c].T
        y1c = Y1[idx_c]                                   # [n,64]
        y1pad = np.zeros((nb * 128, H), F32)
        y1pad[:n] = y1c
        im["y1r"][:, bo:bo + nb, :] = y1pad.reshape(nb, 128, H).transpose(1, 0, 2)
        mc = np.zeros(nb * 128, F32)
        mc[:n] = 1.0
        im["mrow"][:, bo:bo + nb] = mc.reshape(nb, 128).T
        pp = 0 if s < 2 else 1
        h = s % 2
        im["mct"][h, PPOFF[pp]:PPOFF[pp] + P] = mc[:P]
        im["icnt"][0, 2 * pp + h] = 1.0 / max(n * H, 1.0)

    trace = bool(os.environ.get("KG_TRACE"))
    if trace:
        import importlib.util
        if importlib.util.find_spec("antenv.axon_hooks") is None:
            trace = False
    res = run_bass_kernel_spmd(nc, in_maps, list(range(NCORE)), trace=trace)
    if trace:
        _LAST_EXEC_NS = res.exec_time_ns
        it = res.instructions_and_trace
        _LAST_TRACE_PATH = it[1] if it else None

    # unpack: compact x^T per graph -> XC [C, 768, 64]
    XC = np.zeros((C, 768, H), F32)
    for (g, r, s) in asg:
        P = SLOTS[s]
        n = int(counts[g])
        pp = 0 if s < 2 else 1
        h = s % 2
        xo = np.asarray(res.results[r]["xo"])
        xT = xo[64 * h:64 * (h + 1), PPOFF[pp]:PPOFF[pp] + P]
        XC[g, :n] = xT[:, :n].T
    return XC, idx_of


# ---------------- public entry --------------------------------------------
def kernel(num_data, cat_data, num_w, num_b, cat_emb, fi_w1, fi_b1, fi_g,
           fi_be, fi_w2, fi_b2, gcn1_w, gcn1_b, gcn2_w, gcn2_b, pw1, pb1,
           pg, pbe, pw2, pb2):
    args = [np.asarray(a) for a in (num_data, cat_data, num_w, num_b, cat_emb,
                                    fi_w1, fi_b1, fi_g, fi_be, fi_w2, fi_b2,
                                    gcn1_w)]
    fe3, idx, mT, pm, Y1 = _front(*args)
    cols = np.sort(idx, axis=1)
    counts = mT.sum(1)
    asg = None if os.environ.get("KG_NUMPY") else _assign(counts)
    gathered = None
    if asg is not None:
        try:
            XC, idx_of = _middle_device(pm, mT, Y1, np.asarray(gcn1_b),
                                        np.asarray(gcn2_w), np.asarray(gcn2_b),
                                        counts, asg)
            pos = (np.cumsum(mT, axis=1) - 1.0).astype(np.int64)   # [C,B]
            gathered = XC[cols, pos[cols, np.arange(B)[:, None]]]  # [B,K,H]
        except Exception as ex:  # safety net: never return garbage
            print(f"[kernel] device path failed ({ex!r}); numpy fallback",
                  file=sys.stderr)
            gathered = None
    if gathered is None:
        xs = _middle_np(pm, mT, Y1, np.asarray(gcn1_b), np.asarray(gcn2_w),
                        np.asarray(gcn2_b))
        gathered = xs[cols, np.arange(B)[:, None]]                 # [B,K,H]
    full = np.concatenate([gathered, fe3], axis=1).reshape(B, (K + C) * H)
    h = _ln_last(np.maximum(full @ np.asarray(pw1) + np.asarray(pb1), 0.0),
                 np.asarray(pg), np.asarray(pbe))
    out = h @ np.asarray(pw2) + np.asarray(pb2)
    return out.astype(F32)


# revision 9
# speedup vs baseline: 1.3820x; 1.3820x over previous
"""Trainium2 kernel for nn_K_graph (gnn_message_passing).

Strategy: per sharding_hint, the C=32 per-column subgraphs are distributed
across 8 NeuronCores (4 per core). Each graph c only contains the rows whose
top-K includes column c (counts range ~16..680, mean 256), so the device
kernel works on COMPACTED per-graph node lists instead of the full B=1024:
graphs are sorted by node count into four size slots [768, 384, 384, 256]
(one graph per slot per core; identical instruction stream on all cores).

Per graph (compact size P, nb = P/128 row blocks):
  S = pmc pmc^T            K=32 matmuls; diagonal suppressed by accumulating
                           a (-BIG*I) @ I matmul into the same PSUM bank
  f = exp(S)               scalar engine, PSUM->SBUF
  E = (f > 1) * f          vector stt (exactly reproduces the S>0 mask since
                           structural zeros give f == 1.0), rowsums via accum
  deg/dinv chain           tiny per-partition ops
  2 GCN layers, transposed layout: u^T[h,i] = sum_j E[j,i] ydn[j,h] computed
  with the small ydn blocks as the stationary matmul operand (E symmetric);
  the self-loop term enters the same PSUM accumulation as ydn^T Z*eye; the
  masked layernorm runs on [128, P] pair-stacked tiles (two graphs share the
  128 partitions, 64 rows of h each).

Host does the tiny front (feature embed, importance MLP, top-K) and tail
(gather + prediction MLP) plus the compaction bookkeeping.
"""
import sys, os
sys.path.insert(0, "/opt/trn_rl_repo")
import numpy as np

B, NN, NC, H, V, K = 1024, 16, 16, 64, 100, 8
C = NN + NC
NEG = -1e9
NCORE = 8
GPC = C // NCORE  # graphs per core = 4

F32 = np.float32

# compact slot layout (identical on every core)
SLOTS = [768, 384, 384, 256]
NBS = [6, 3, 3, 2]
BLKOFF = [0, 6, 9, 12]
NB_TOT = 14
COLOFF = [0, 768, 1152, 1536]
P_TOT = 1792
PAIRS = [(0, 1), (2, 3)]     # (bigger slot -> partitions 0:64, smaller -> 64:128)
PPOFF = [0, 768]             # column offset of each pair in mct / xo
PO_TOT = 1152
BIG = 50.0


# ---------------- host front (numpy mirror of reference front) -------------
def _ln_all(x, eps=1e-5):
    mu = x.mean()
    var = ((x - mu) ** 2).mean()
    return (x - mu) / np.sqrt(var + eps)


def _ln_last(x, g, b, eps=1e-5):
    mu = x.mean(-1, keepdims=True)
    var = ((x - mu) ** 2).mean(-1, keepdims=True)
    return (x - mu) / np.sqrt(var + eps) * g + b


def _front(num_data, cat_data, num_w, num_b, cat_emb, fi_w1, fi_b1, fi_g,
           fi_be, fi_w2, fi_b2, gcn1_w):
    fe_num = num_data[..., None] * num_w[None] + num_b[None]
    fe_num = _ln_all(np.maximum(fe_num.reshape(B, NN * H), 0.0))
    fe_cat = cat_emb[np.arange(NC)[None, :], cat_data]
    fe_cat = _ln_all(fe_cat.reshape(B, NC * H))
    feat = np.concatenate([fe_num, fe_cat], axis=1).astype(F32)
    fe3 = feat.reshape(B, C, H)
    h = np.maximum(fe3 @ fi_w1 + fi_b1, 0.0)
    h = _ln_last(h, fi_g, fi_be)
    imp = _ln_all((h @ fi_w2 + fi_b2)[..., 0]).astype(F32)   # [B,C]
    fe3 = (fe3 * imp[..., None]).astype(F32)
    feat = fe3.reshape(B, C * H)
    # top-K per row
    idx = np.argsort(-imp, axis=1, kind="stable")[:, :K]      # [B,K]
    mask = np.zeros((B, C), F32)
    np.put_along_axis(mask, idx, 1.0, axis=1)
    z = np.where(mask > 0, imp, NEG)
    z = z - z.max(1, keepdims=True)
    e = np.exp(z)
    p = (e / e.sum(1, keepdims=True)) * mask                  # [B,C]
    mT = mask.T.copy()                                        # [C,B]
    pm = p[None, :, :] * mT[:, :, None] * (1.0 - np.eye(C, dtype=F32))[:, None, :]
    Y1 = (feat @ gcn1_w).astype(F32)                          # [B,H]
    return fe3, idx, mT, pm.astype(F32), Y1


# ---------------- numpy middle (validation / fallback) ---------------------
def _middle_np(pm, mT, Y1, gcn1_b, gcn2_w, gcn2_b):
    xs = np.zeros((C, B, H), F32)
    for c in range(C):
        M = pm[c]                               # [B,C]
        S = (M @ M.T) * (1.0 - np.eye(B, dtype=F32))
        Ffull = np.exp(S)
        E = (S > 0).astype(F32) * Ffull
        rs = E.sum(1)
        Z = rs.sum()
        Zg = Z + (1.0 if Z <= 0 else 0.0)
        invZ = 1.0 / Zg
        m = mT[c]
        deg = rs * invZ + m
        dinv = 1.0 / np.sqrt(deg + 1.0 - m) * m
        x = Y1
        for (W, bvec) in ((None, gcn1_b), (gcn2_w, gcn2_b)):
            Yin = x if W is None else x @ W
            Ydn = dinv[:, None] * Yin
            u = E @ Ydn
            xl = dinv[:, None] * (u * invZ + m[:, None] * Ydn) + bvec
            r = np.maximum(xl, 0.0)
            rm = r * m[:, None]
            cnt = max(m.sum() * H, 1.0)
            mu = rm.sum() / cnt
            var = (rm * rm).sum() / cnt - mu * mu
            x = (r - mu) / np.sqrt(var + 1e-5)
        xs[c] = x
    return xs


# ---------------- device kernel -------------------------------------------
def _build_device():
    from concourse import bacc, tile
    import concourse.bass as bass
    import concourse.mybir as mybir
    dt = mybir.dt.float32
    db = mybir.dt.bfloat16
    ALU = mybir.AluOpType
    ACT = mybir.ActivationFunctionType
    AX = mybir.AxisListType

    nc = bacc.Bacc(None, target_bir_lowering=False, debug=False)
    pmT_d = nc.declare_dram_parameter("pmT", [32, P_TOT], db, isOutput=False)
    y1r_d = nc.declare_dram_parameter("y1r", [128, NB_TOT, H], dt, isOutput=False)
    mrow_d = nc.declare_dram_parameter("mrow", [128, NB_TOT], dt, isOutput=False)
    mct_d = nc.declare_dram_parameter("mct", [2, PO_TOT], db, isOutput=False)
    icnt_d = nc.declare_dram_parameter("icnt", [1, 4], dt, isOutput=False)
    w2_d = nc.declare_dram_parameter("w2rep", [128, H], db, isOutput=False)
    b1_d = nc.declare_dram_parameter("b1bc", [128, 1], dt, isOutput=False)
    b2_d = nc.declare_dram_parameter("b2bc", [128, 1], dt, isOutput=False)
    eye_d = nc.declare_dram_parameter("eye", [128, 128], dt, isOutput=False)
    e2_d = nc.declare_dram_parameter("e2", [2, 128], db, isOutput=False)
    e2t_d = nc.declare_dram_parameter("e2t", [128, 2], dt, isOutput=False)
    sel_d = nc.declare_dram_parameter("sel", [6, 6 * H], db, isOutput=False)
    xo_d = nc.declare_dram_parameter("xo", [128, PO_TOT], db, isOutput=True)

    def chunks_of(P):
        return [(c, min(c + 512, P)) for c in range(0, P, 512)]

    with tile.TileContext(nc) as tc:
        with (
            tc.tile_pool(name="const", bufs=1) as cpool,
            tc.tile_pool(name="estore", bufs=1) as epool,
            tc.tile_pool(name="work", bufs=2) as wp,
            tc.tile_pool(name="scal", bufs=3) as sp,
            tc.tile_pool(name="psS", bufs=2, space=bass.MemorySpace.PSUM) as psS,
            tc.tile_pool(name="psU", bufs=1, space=bass.MemorySpace.PSUM) as psU,
            tc.tile_pool(name="psT", bufs=2, space=bass.MemorySpace.PSUM) as psT,
        ):
            pmT_sb = cpool.tile([32, P_TOT], db)
            y1r_sb = cpool.tile([128, NB_TOT, H], dt)
            mrow_sb = cpool.tile([128, NB_TOT], dt)
            mct_sb = cpool.tile([2, PO_TOT], db)
            icnt_sb = cpool.tile([1, 4], dt)
            w2_sb = cpool.tile([128, H], db)
            b1_sb = cpool.tile([128, 1], dt)
            b2_sb = cpool.tile([128, 1], dt)
            eye_sb = cpool.tile([128, 128], dt)
            e2_sb = cpool.tile([2, 128], db)
            e2t_sb = cpool.tile([128, 2], dt)
            sel_sb = cpool.tile([6, 6 * H], db)
            negeye = cpool.tile([128, 128], db)
            eye_bf = cpool.tile([128, 128], db)
            ones_r = cpool.tile([1, 128], dt)
            ones_rb = cpool.tile([1, 128], db)
            ones_c = cpool.tile([128, 1], dt)
            nc.sync.dma_start(pmT_sb[:], pmT_d[:])
            nc.sync.dma_start(y1r_sb[:], y1r_d[:])
            nc.sync.dma_start(mrow_sb[:], mrow_d[:])
            nc.sync.dma_start(mct_sb[:], mct_d[:])
            nc.sync.dma_start(icnt_sb[:], icnt_d[:])
            nc.sync.dma_start(w2_sb[:], w2_d[:])
            nc.sync.dma_start(b1_sb[:], b1_d[:])
            nc.sync.dma_start(b2_sb[:], b2_d[:])
            nc.sync.dma_start(eye_sb[:], eye_d[:])
            nc.sync.dma_start(e2_sb[:], e2_d[:])
            nc.sync.dma_start(e2t_sb[:], e2t_d[:])
            nc.sync.dma_start(sel_sb[:], sel_d[:])
            nc.vector.memset(ones_r[:], 1.0)
            nc.vector.memset(ones_rb[:], 1.0)
            nc.vector.memset(ones_c[:], 1.0)
            nc.vector.tensor_scalar_mul(negeye[:], eye_sb[:], -BIG)
            nc.vector.tensor_copy(eye_bf[:], eye_sb[:])

            def bscalar(src_11, tag):
                """broadcast [1,1] sbuf scalar -> [128,1] sbuf"""
                sb16 = sp.tile([1, 1], db, tag="b16")
                nc.vector.tensor_copy(sb16[:], src_11)
                ps = psT.tile([128, 1], dt, tag="sm")
                nc.tensor.matmul(ps[:], ones_rb[:], sb16[:], start=True,
                                 stop=True)
                sb = sp.tile([128, 1], dt, tag=tag)
                nc.vector.tensor_copy(sb[:], ps[:])
                return sb

            def bpair(src_12, tag):
                """[1,2] sbuf -> [128,1] sbuf with halves from cols 0/1"""
                sb16 = sp.tile([1, 2], db, tag="p16")
                nc.vector.tensor_copy(sb16[:], src_12)
                ps1 = psT.tile([2, 128], dt, tag="sm")
                nc.tensor.matmul(ps1[:], sb16[:], ones_rb[:], start=True,
                                 stop=True)
                v2r = sp.tile([2, 128], db, tag="v2r")
                nc.vector.tensor_copy(v2r[:], ps1[:])
                ps2 = psT.tile([128, 1], dt, tag="sm")
                nc.tensor.matmul(ps2[:], e2_sb[:], v2r[:, 0:1], start=True,
                                 stop=True)
                sb = sp.tile([128, 1], dt, tag=tag)
                nc.vector.tensor_copy(sb[:], ps2[:])
                return sb

            # ---------------- per-slot: S, E, rowsums ----------------
            E_sb = []
            rs_sb = []
            for s in range(4):
                P, nb, off = SLOTS[s], NBS[s], COLOFF[s]
                E_s = epool.tile([128, nb, P], db, tag=f"E{s}")
                rs_s = wp.tile([128, nb], dt, tag=f"rs{s}")
                for ib in range(nb):
                    s_ps = psS.tile([128, 768], dt, tag="sps")
                    d0 = ib * 128
                    for (c0, c1) in chunks_of(P):
                        has_diag = c0 <= d0 < c1
                        nc.tensor.matmul(
                            s_ps[:, c0:c1],
                            pmT_sb[:, off + d0:off + d0 + 128],
                            pmT_sb[:, off + c0:off + c1],
                            start=True, stop=not has_diag)
                        if has_diag:
                            nc.tensor.matmul(
                                s_ps[:, d0:d0 + 128], negeye[:], eye_bf[:],
                                start=False, stop=True, skip_group_check=True)
                    f_sb = wp.tile([128, 768], dt, tag="f")
                    nc.scalar.activation(f_sb[:, 0:P], s_ps[:, 0:P], ACT.Exp)
                    nc.vector.scalar_tensor_tensor(
                        E_s[:, ib, :], f_sb[:, 0:P], 1.0, f_sb[:, 0:P],
                        ALU.is_gt, ALU.mult,
                        accum_out=rs_s[:, ib:ib + 1])
                E_sb.append(E_s)
                rs_sb.append(rs_s)

            # ---------------- per-pair processing ----------------
            for pp, (sa, sb_) in enumerate(PAIRS):
                PA, PB = SLOTS[sa], SLOTS[sb_]
                nbA, nbB = NBS[sa], NBS[sb_]
                poff = PPOFF[pp]

                # Z totals -> [1,2]
                z_ps = psT.tile([1, 2], dt, tag="sm")
                for h, s in enumerate((sa, sb_)):
                    rsr = sp.tile([128, 1], dt, tag="rsr")
                    nc.vector.tensor_reduce(rsr[:], rs_sb[s][:], AX.X, ALU.add)
                    nc.tensor.matmul(z_ps[:, h:h + 1], rsr[:], ones_c[:],
                                     start=True, stop=True,
                                     skip_group_check=True)
                z2 = sp.tile([1, 2], dt, tag="z2")
                nc.vector.tensor_copy(z2[:], z_ps[:])
                zi = sp.tile([1, 2], dt, tag="zi")
                nc.vector.tensor_scalar(zi[:], z2[:], 0.0, None, ALU.is_le)
                zg = sp.tile([1, 2], dt, tag="zg")
                nc.vector.tensor_add(zg[:], z2[:], zi[:])
                invz2 = sp.tile([1, 2], dt, tag="invz2")
                nc.vector.reciprocal(invz2[:], zg[:])

                invzP = [bscalar(invz2[:, h:h + 1], f"invzP{pp}_{h}")
                         for h in range(2)]
                zgP = [bscalar(zg[:, h:h + 1], f"zgP{pp}_{h}")
                       for h in range(2)]

                # deg chain per graph (row layout) + eyeZ + dinvT broadcast
                dinv_g = []
                eyeZ_g = []
                dbc = wp.tile([128, 768 if pp == 0 else 384], dt,
                              tag=f"dbc{pp}")
                dbc_ps = psU.tile([128, 768], dt, tag="ubig")
                for h, s in enumerate((sa, sb_)):
                    P, nb = SLOTS[s], NBS[s]
                    degg = wp.tile([128, NB_TOT], dt, tag="degg")
                    nc.scalar.activation(degg[:, 0:nb], rs_sb[s][:],
                                         ACT.Identity, bias=1.0,
                                         scale=invzP[h][:, 0:1])
                    dsq = wp.tile([128, NB_TOT], dt, tag="dsq")
                    nc.scalar.activation(dsq[:, 0:nb], degg[:, 0:nb], ACT.Sqrt)
                    draw = wp.tile([128, NB_TOT], dt, tag="draw")
                    nc.vector.reciprocal(draw[:, 0:nb], dsq[:, 0:nb])
                    dinv = wp.tile([128, NB_TOT], dt, tag=f"dinv{pp}_{h}")
                    nc.vector.tensor_mul(
                        dinv[:, 0:nb], draw[:, 0:nb],
                        mrow_sb[:, BLKOFF[s]:BLKOFF[s] + nb])
                    dinv_g.append(dinv)
                    eyeZ = wp.tile([128, 128], db, tag=f"eyeZ{pp}_{h}")
                    nc.vector.tensor_scalar_mul(eyeZ[:], eye_bf[:],
                                                zgP[h][:, 0:1])
                    eyeZ_g.append(eyeZ)
                    # dinv [128,nb] -> transposed broadcast rows in dbc half
                    t_ps = psT.tile([6, 128], dt, tag="sm")
                    nc.tensor.transpose(t_ps[0:nb, :], dinv[:, 0:nb], eye_sb[:])
                    dT = sp.tile([6, 128], db, tag="dT")
                    nc.vector.tensor_copy(dT[0:nb, :], t_ps[0:nb, :])
                    for jb in range(nb):
                        nc.tensor.matmul(
                            dbc_ps[h * 64:(h + 1) * 64, jb * 128:(jb + 1) * 128],
                            sel_sb[0:nb, jb * H:(jb + 1) * H],
                            dT[0:nb, :], start=True, stop=True,
                            skip_group_check=True)
                nc.scalar.activation(dbc[0:64, 0:PA], dbc_ps[0:64, 0:PA],
                                     ACT.Copy)
                nc.scalar.activation(dbc[64:128, 0:PB], dbc_ps[64:128, 0:PB],
                                     ACT.Copy)
                if PB < PA:
                    nc.vector.memset(dbc[64:128, PB:PA], 0.0)

                # mask broadcast tile for the pair
                mbc = wp.tile([128, 768 if pp == 0 else 384], dt,
                              tag=f"mbc{pp}")
                mbc_ps = psU.tile([128, 768], dt, tag="ubig")
                for (c0, c1) in chunks_of(PA):
                    nc.tensor.matmul(mbc_ps[:, c0:c1], e2_sb[:],
                                     mct_sb[:, poff + c0:poff + c1],
                                     start=True, stop=True,
                                     skip_group_check=True)
                nc.scalar.activation(mbc[:, 0:PA], mbc_ps[:, 0:PA], ACT.Copy)

                # ---------------- two GCN layers ----------------
                x_prev = None
                for layer in range(2):
                    b_sb = b1_sb if layer == 0 else b2_sb
                    # ydn row blocks [128, nb, H] per graph
                    ydn = wp.tile([128, NB_TOT, H], db, tag=f"ydn{pp}")
                    for h, s in enumerate((sa, sb_)):
                        nb, bo = NBS[s], BLKOFF[s]
                        for jb in range(nb):
                            if layer == 0:
                                nc.vector.tensor_scalar_mul(
                                    ydn[:, bo + jb, :], y1r_sb[:, bo + jb, :],
                                    dinv_g[h][:, jb:jb + 1])
                            else:
                                y2_ps = psT.tile([128, H], dt, tag="sm")
                                nc.tensor.matmul(
                                    y2_ps[:],
                                    x_prev[h * 64:(h + 1) * 64,
                                           jb * 128:(jb + 1) * 128],
                                    w2_sb[h * 64:(h + 1) * 64, :],
                                    start=True, stop=True)
                                nc.scalar.activation(
                                    ydn[:, bo + jb, :], y2_ps[:], ACT.Copy,
                                    scale=dinv_g[h][:, jb:jb + 1])
                    # propagation matmuls: u^T += E^T ydn + Z ydn^T
                    u_ps = psU.tile([128, 768], dt, tag="ubig")
                    for h, s in enumerate((sa, sb_)):
                        P, nb, bo = SLOTS[s], NBS[s], BLKOFF[s]
                        h0 = h * 64
                        for (c0, c1) in chunks_of(P):
                            for jb in range(nb):
                                nc.tensor.matmul(
                                    u_ps[h0:h0 + 64, c0:c1],
                                    ydn[:, bo + jb, :],
                                    E_sb[s][:, jb, c0:c1],
                                    start=(jb == 0), stop=False,
                                    skip_group_check=True)
                            n_ib = [ib for ib in range(nb)
                                    if c0 <= ib * 128 < c1]
                            for k, ib in enumerate(n_ib):
                                nc.tensor.matmul(
                                    u_ps[h0:h0 + 64, ib * 128:(ib + 1) * 128],
                                    ydn[:, bo + ib, :], eyeZ_g[h][:],
                                    start=False, stop=(k == len(n_ib) - 1),
                                    skip_group_check=True)
                    # t = invz * u   (junk tail of smaller graph zeroed)
                    t_sb = wp.tile([128, 768 if pp == 0 else 384], dt,
                                   tag=f"t{pp}")
                    nc.scalar.activation(t_sb[0:64, 0:PA], u_ps[0:64, 0:PA],
                                         ACT.Copy, scale=invzP[0][0:64, 0:1])
                    nc.scalar.activation(t_sb[64:128, 0:PB],
                                         u_ps[64:128, 0:PB],
                                         ACT.Copy, scale=invzP[1][64:128, 0:1])
                    if PB < PA:
                        nc.vector.memset(t_sb[64:128, PB:PA], 0.0)
                    w_sb = wp.tile([128, 768 if pp == 0 else 384], dt,
                                   tag=f"w{pp}")
                    nc.vector.tensor_mul(w_sb[:, 0:PA], t_sb[:, 0:PA],
                                         dbc[:, 0:PA])
                    r_sb = wp.tile([128, 768 if pp == 0 else 384], dt,
                                   tag=f"r{pp}")
                    nc.vector.tensor_scalar(r_sb[:, 0:PA], w_sb[:, 0:PA],
                                            b_sb[:, 0:1], 0.0, ALU.add,
                                            ALU.max)
                    s1 = sp.tile([128, 1], dt, tag="s1")
                    rm_sb = wp.tile([128, 768 if pp == 0 else 384], dt,
                                    tag=f"rm{pp}")
                    nc.vector.scalar_tensor_tensor(
                        rm_sb[:, 0:PA], r_sb[:, 0:PA], 1.0, mbc[:, 0:PA],
                        ALU.mult, ALU.mult, accum_out=s1[:])
                    s2 = sp.tile([128, 1], dt, tag="s2")
                    sq_sb = wp.tile([128, 768 if pp == 0 else 384], dt,
                                    tag=f"sq{pp}")
                    nc.scalar.activation(sq_sb[:, 0:PA], rm_sb[:, 0:PA],
                                         ACT.Square, accum_out=s2[:])
                    # per-graph stats: [1,2] = per-half partition sums
                    st_ps = psT.tile([1, 4], dt, tag="sm")
                    nc.tensor.matmul(st_ps[:, 0:2], s1[:], e2t_sb[:],
                                     start=True, stop=True,
                                     skip_group_check=True)
                    nc.tensor.matmul(st_ps[:, 2:4], s2[:], e2t_sb[:],
                                     start=True, stop=True,
                                     skip_group_check=True)
                    st = sp.tile([1, 4], dt, tag="st")
                    nc.vector.tensor_copy(st[:], st_ps[:])
                    mu = sp.tile([1, 2], dt, tag="mu")
                    nc.vector.tensor_mul(mu[:], st[:, 0:2],
                                         icnt_sb[:, 2 * pp:2 * pp + 2])
                    e2m = sp.tile([1, 2], dt, tag="e2m")
                    nc.vector.tensor_mul(e2m[:], st[:, 2:4],
                                         icnt_sb[:, 2 * pp:2 * pp + 2])
                    musq = sp.tile([1, 2], dt, tag="musq")
                    nc.vector.tensor_mul(musq[:], mu[:], mu[:])
                    var = sp.tile([1, 2], dt, tag="var")
                    nc.vector.tensor_sub(var[:], e2m[:], musq[:])
                    vare = sp.tile([1, 2], dt, tag="vare")
                    nc.vector.tensor_scalar_add(vare[:], var[:], 1e-5)
                    sig = sp.tile([1, 2], dt, tag="sig")
                    nc.scalar.activation(sig[:], vare[:], ACT.Sqrt)
                    rsig = sp.tile([1, 2], dt, tag="rsig")
                    nc.vector.reciprocal(rsig[:], sig[:])
                    nmr = sp.tile([1, 2], dt, tag="nmr")
                    nc.vector.scalar_tensor_tensor(nmr[:], mu[:], -1.0,
                                                   rsig[:], ALU.mult, ALU.mult)
                    rsigP = bpair(rsig[:], f"rsigP{pp}")
                    nmrP = bpair(nmr[:], f"nmrP{pp}")
                    x_sb = wp.tile([128, 768 if pp == 0 else 384], db,
                                   tag=f"x{layer}_{pp}")
                    nc.vector.tensor_scalar(x_sb[:, 0:PA], r_sb[:, 0:PA],
                                            rsigP[:, 0:1], nmrP[:, 0:1],
                                            ALU.mult, ALU.add)
                    x_prev = x_sb
                nc.sync.dma_start(xo_d[:, poff:poff + PA], x_prev[:, 0:PA])
    nc.finalize()
    return nc


_NC_CACHE = None
_LAST_EXEC_NS = None
_LAST_TRACE_PATH = None


def _assign(counts):
    """sort graphs into (core, slot); returns list of (graph, core, slot)."""
    order = np.argsort(-counts, kind="stable")
    for s in range(4):
        if counts[order[8 * s]] > SLOTS[s]:
            return None
    asg = []
    for s in range(4):
        for r in range(NCORE):
            asg.append((int(order[8 * s + r]), r, s))
    return asg


def _middle_device(pm, mT, Y1, gcn1_b, gcn2_w, gcn2_b, counts, asg):
    global _NC_CACHE, _LAST_EXEC_NS, _LAST_TRACE_PATH
    from concourse.bass_utils import run_bass_kernel_spmd
    import ml_dtypes
    BF16 = ml_dtypes.bfloat16
    if _NC_CACHE is None:
        _NC_CACHE = _build_device()
    nc = _NC_CACHE

    eye = np.eye(128, dtype=F32)
    e2 = np.zeros((2, 128), F32)
    e2[0, 0:64] = 1.0
    e2[1, 64:128] = 1.0
    e2t = e2.T.copy()
    sel = np.zeros((6, 6 * H), F32)
    for jb in range(6):
        sel[jb, jb * H:(jb + 1) * H] = 1.0
    w2rep = np.concatenate([gcn2_w, gcn2_w], axis=0).astype(BF16)  # [128,64]
    b1bc = np.concatenate([gcn1_b, gcn1_b])[:, None].astype(F32)  # [128,1]
    b2bc = np.concatenate([gcn2_b, gcn2_b])[:, None].astype(F32)

    idx_of = {}
    in_maps = []
    for r in range(NCORE):
        in_maps.append(dict(
            pmT=np.zeros((32, P_TOT), BF16),
            y1r=np.zeros((128, NB_TOT, H), F32),
            mrow=np.zeros((128, NB_TOT), F32),
            mct=np.zeros((2, PO_TOT), BF16),
            icnt=np.ones((1, 4), F32),
            w2rep=w2rep, b1bc=b1bc, b2bc=b2bc, eye=eye,
            e2=e2.astype(BF16), e2t=e2t, sel=sel.astype(BF16)))
    for (g, r, s) in asg:
        P, nb, bo, co = SLOTS[s], NBS[s], BLKOFF[s], COLOFF[s]
        n = int(counts[g])
        idx_c = np.nonzero(mT[g] > 0)[0]
        idx_of[g] = idx_c
        im = in_maps[r]
        im["pmT"][:, co:co + n] = pm[g][idx_c].T
        y1c = Y1[idx_c]                                   # [n,64]
        y1pad = np.zeros((nb * 128, H), F32)
        y1pad[:n] = y1c
        im["y1r"][:, bo:bo + nb, :] = y1pad.reshape(nb, 128, H).transpose(1, 0, 2)
        mc = np.zeros(nb * 128, F32)
        mc[:n] = 1.0
        im["mrow"][:, bo:bo + nb] = mc.reshape(nb, 128).T
        pp = 0 if s < 2 else 1
        h = s % 2
        im["mct"][h, PPOFF[pp]:PPOFF[pp] + P] = mc[:P]
        im["icnt"][0, 2 * pp + h] = 1.0 / max(n * H, 1.0)

    trace = bool(os.environ.get("KG_TRACE"))
    if trace:
        import importlib.util
        if importlib.util.find_spec("antenv.axon_hooks") is None:
            trace = False
    res = run_bass_kernel_spmd(nc, in_maps, list(range(NCORE)), trace=trace)
    if trace:
        _LAST_EXEC_NS = res.exec_time_ns
        it = res.instructions_and_trace
        _LAST_TRACE_PATH = it[1] if it else None

    # unpack: compact x^T per graph -> XC [C, 768, 64]
    XC = np.zeros((C, 768, H), F32)
    for (g, r, s) in asg:
        P = SLOTS[s]
        n = int(counts[g])
        pp = 0 if s < 2 else 1
        h = s % 2
        xo = np.asarray(res.results[r]["xo"]).astype(F32)
        xT = xo[64 * h:64 * (h + 1), PPOFF[pp]:PPOFF[pp] + P]
        XC[g, :n] = xT[:, :n].T
    return XC, idx_of


# ---------------- public entry --------------------------------------------
def kernel(num_data, cat_data, num_w, num_b, cat_emb, fi_w1, fi_b1, fi_g,
           fi_be, fi_w2, fi_b2, gcn1_w, gcn1_b, gcn2_w, gcn2_b, pw1, pb1,
           pg, pbe, pw2, pb2):
    args = [np.asarray(a) for a in (num_data, cat_data, num_w, num_b, cat_emb,
                                    fi_w1, fi_b1, fi_g, fi_be, fi_w2, fi_b2,
                                    gcn1_w)]
    fe3, idx, mT, pm, Y1 = _front(*args)
    cols = np.sort(idx, axis=1)
    counts = mT.sum(1)
    asg = None if os.environ.get("KG_NUMPY") else _assign(counts)
    gathered = None
    if asg is not None:
        try:
            XC, idx_of = _middle_device(pm, mT, Y1, np.asarray(gcn1_b),
                                        np.asarray(gcn2_w), np.asarray(gcn2_b),
                                        counts, asg)
            pos = (np.cumsum(mT, axis=1) - 1.0).astype(np.int64)   # [C,B]
            gathered = XC[cols, pos[cols, np.arange(B)[:, None]]]  # [B,K,H]
        except Exception as ex:  # safety net: never return garbage
            print(f"[kernel] device path failed ({ex!r}); numpy fallback",
                  file=sys.stderr)
            gathered = None
    if gathered is None:
        xs = _middle_np(pm, mT, Y1, np.asarray(gcn1_b), np.asarray(gcn2_w),
                        np.asarray(gcn2_b))
        gathered = xs[cols, np.arange(B)[:, None]]                 # [B,K,H]
    full = np.concatenate([gathered, fe3], axis=1).reshape(B, (K + C) * H)
    h = _ln_last(np.maximum(full @ np.asarray(pw1) + np.asarray(pb1), 0.0),
                 np.asarray(pg), np.asarray(pbe))
    out = h @ np.asarray(pw2) + np.asarray(pb2)
    return out.astype(F32)
